# revision 24
# baseline (speedup 1.0000x reference)
"""Multi-head causal attention (B=2, L=2048, D=1024, H=16) on 8 trn2 cores.

Sharding: data-parallel over batch (2) x tensor-parallel over heads (4 groups
of 4 heads).  Core c handles batch c//4, heads 4*(c%4) .. 4*(c%4)+3.
Wq/Wk/Wv are column-sharded, Wo row-sharded; the TP all-reduce after Wo is
done host-side (sum of the 4 partial outputs per batch), as is the bo add.

Per-core kernel:
  - QKV projections run in fp8e4 DoubleRow perf mode: the host supplies
    X^T split into e4m3 hi/lo residual streams (same total bytes as bf16)
    and W scaled by 256 split hi/lo; the three products hi*hi + hi*lo +
    lo*hi are computed with DoubleRow matmuls (two 128-contraction chunks
    per instruction at 0.5 cycles/row), 25% cheaper than bf16 and at
    bf16-level accuracy.  The 1/256 unscale + bias folds into the
    PSUM->SBUF copy.
  - Q^T/K^T are quantized to e4m3 and shuffled (SBUF->SBUF DMA) into a
    [32, 2(dk-half), head, n] layout so S^T = K^T tile ^T Q^T runs as a
    single fp8 DoubleRow matmul per (key-tile, head) - half the bf16 cost.
  - softmax without max-subtraction (scores bounded); exp on ACT emits
    P^T bf16 directly in the layout PV needs; a ones-row appended to V
    makes the PV matmul also emit softmax denominators
  - normalization: reciprocal of the denom row, partition-broadcast via a
    K=1 matmul (ones x recip), multiplied into O^T before the Wo matmul
  - causal mask: S^T/exp/PV restricted to q >= k-tile start; the single
    diagonal 128x128 block is masked by multiplying P^T with an upper-
    triangular 0/1 tile (supplied as input)
  - two heads are processed per S^T psum tile so each exp instruction
    covers [128, 2, 512-lo] (ACT per-instruction overhead is ~370ns)
"""

import numpy as np

B, L, D, H = 2, 2048, 1024, 16
DK = D // H          # 64
NCORES = 8
TP = 4               # head-group shards per batch
HG = H // TP         # 4 heads per core
DH = HG * DK         # 256 per-core head dims
STRIP = 512          # attention q-strip width
NSTRIP = L // STRIP  # 4
SUB = 512            # projection substrip width
NSUB = L // SUB      # 4
KT = 128             # key tile
NKT = L // KT        # 16
WS = 256.0           # fp8 weight pre-scale (undone in PSUM->SBUF copies)

_CACHE = {}


def _build(causal: bool, qkv_bias: bool = True):
    import concourse.mybir as mybir
    import concourse.tile as tile
    from concourse import bacc

    f32 = mybir.dt.float32
    f32r = mybir.dt.float32r
    bf16 = mybir.dt.bfloat16
    fp8 = mybir.dt.float8e4
    f16 = mybir.dt.float16
    EXP = mybir.ActivationFunctionType.Exp
    DR = mybir.MatmulPerfMode.DoubleRow
    MULT = mybir.AluOpType.mult
    ADD = mybir.AluOpType.add

    nc = bacc.Bacc("TRN2", target_bir_lowering=False)

    # hi/lo fp8 input streams (X^T, [D, L])
    xs = {}
    for nm in ("q", "k", "v"):
        xs[nm + "h"] = nc.dram_tensor(nm + "Th", [D, L], fp8, kind="ExternalInput")
        xs[nm + "l"] = nc.dram_tensor(nm + "Tl", [D, L], fp8, kind="ExternalInput")
    # weights: wh = e4m3(W*WS) [D, DH]; wx = (lo, hi) interleaved [D, 2*DH]
    ws = {}
    for nm in ("wq", "wk", "wv"):
        ws[nm + "h"] = nc.dram_tensor(nm + "h", [128, 8 * DH], fp8, kind="ExternalInput")
        ws[nm + "x"] = nc.dram_tensor(nm + "x", [D, 2 * DH], fp8, kind="ExternalInput")
        
    wo = nc.dram_tensor("wo", [DH, D], bf16, kind="ExternalInput")
    bq = nc.dram_tensor("bq", [DH], f32, kind="ExternalInput")
    bk = nc.dram_tensor("bk", [DH], f32, kind="ExternalInput")
    bv = nc.dram_tensor("bv", [DH], bf16, kind="ExternalInput")  # pre-scaled by WS
    tri = nc.dram_tensor("tri", [KT, KT], bf16, kind="ExternalInput")
    maskT = None
    if not causal:
        maskT = nc.dram_tensor("maskT", [L, L], bf16, kind="ExternalInput")
    out = nc.dram_tensor("out", [L, D], f16, kind="ExternalOutput")

    with tile.TileContext(nc) as tc:
        with (
            tc.tile_pool(name="consts", bufs=1) as consts,
            tc.tile_pool(name="resident", bufs=1) as resident,
            tc.tile_pool(name="xin", bufs=2) as xin,
            tc.tile_pool(name="qtmp", bufs=2) as qtmp,
            tc.tile_pool(name="ptile", bufs=18) as ptile,
            tc.tile_pool(name="small", bufs=2) as small,
            tc.tile_pool(name="stage", bufs=2) as stage,
            tc.tile_pool(name="mtile", bufs=4) as mtile,
            tc.tile_pool(name="bank", bufs=4, space="PSUM") as bank,
            tc.tile_pool(name="sps", bufs=2, space="PSUM") as sps,
        ):
            # ---- input substrip tiles: [128, 8(c), 2(hi/lo), SUB] fp8 ----
            def load_x(nm, t, first=None):
                if first is not None:
                    x_t = first
                else:
                    x_t = xin.tile([128, 8, 2, SUB], fp8, tag="x" + nm)
                sl = slice(t * SUB, (t + 1) * SUB)
                nc.sync.dma_start(out=x_t[:, :, 0, :], in_=xs[nm + "h"][:, sl].rearrange("(c p) n -> p c n", p=128))
                nc.sync.dma_start(out=x_t[:, :, 1, :], in_=xs[nm + "l"][:, sl].rearrange("(c p) n -> p c n", p=128))
                return x_t

            # startup ordering: hi-stream weights + activations first (the
            # hi*hi products can start on them alone), lo streams after
            w_h, w_x = {}, {}
            x_first = {}
            for nm, xnm in (("wq", "q"), ("wk", "k"), ("wv", "v")):
                w_h[nm] = consts.tile([128, 8, DH], fp8, tag=nm + "h", name=nm + "h")
                w_x[nm] = consts.tile([128, 8, 2, DH], fp8, tag=nm + "x", name=nm + "x")
                xt = xin.tile([128, 8, 2, SUB], fp8, tag="x" + xnm, name=f"x{xnm}_pre0")
                x_first[(xnm, 0)] = xt
            def _ldw(nm, xnm, part):
                if part == 0:
                    nc.sync.dma_start(out=w_h[nm], in_=ws[nm + "h"].rearrange("p (c m) -> p c m", c=8))
                    nc.sync.dma_start(out=x_first[(xnm, 0)][:, :, 0, :],
                                      in_=xs[xnm + "h"][:, 0:SUB].rearrange("(c p) n -> p c n", p=128))
                else:
                    nc.sync.dma_start(out=w_x[nm], in_=ws[nm + "x"].rearrange("(c p) (u m) -> p c u m", p=128, u=2))
                    nc.sync.dma_start(out=x_first[(xnm, 0)][:, :, 1, :],
                                      in_=xs[xnm + "l"][:, 0:SUB].rearrange("(c p) n -> p c n", p=128))

            _ldw("wq", "q", 0); _ldw("wk", "k", 0)
            _ldw("wk", "k", 1)
            nc.sync.dma_start(out=w_x["wq"], in_=ws["wqx"].rearrange("(c p) (u m) -> p c u m", p=128, u=2))
            _ldw("wv", "v", 0); _ldw("wv", "v", 1)
            nc.sync.dma_start(out=x_first[("q", 0)][:, :, 1, :],
                              in_=xs["ql"][:, 0:SUB].rearrange("(c p) n -> p c n", p=128))

            bqP = consts.tile([128, 2], f32, tag="bqP")
            bkP = consts.tile([128, 2], f32, tag="bkP")
            nc.sync.dma_start(out=bqP, in_=bq.rearrange("(m p) -> p m", p=128))
            nc.sync.dma_start(out=bkP, in_=bk.rearrange("(m p) -> p m", p=128))
            bv_t = consts.tile([1, DH], bf16, tag="bv")
            nc.sync.dma_start(out=bv_t, in_=bv[:].unsqueeze(0))
            tri_t = consts.tile([KT, KT], bf16, tag="tri")
            nc.sync.dma_start(out=tri_t, in_=tri[:])
            ones_f = consts.tile([1, 128], f32, tag="ones")
            nc.vector.memset(ones_f, 1.0)
            ones_r = ones_f[:, :].bitcast(f32r)
            ones_b = consts.tile([1, 128], bf16, tag="onesb")
            nc.vector.memset(ones_b, 1.0)
            wo_t = consts.tile([128, 2, D], bf16, tag="wo")
            nc.sync.dma_start(out=wo_t, in_=wo.rearrange("(c p) n -> p c n", p=128))

            # ---- resident activations ----
            # q8: [128(=head-half*64+dk), 2(m-chunk), 2(hi/lo), n] fp8
            # k8: [128, 2(m-chunk), n] fp8 (single)
            q_s, k_s, v_s, o_s = [], [], [], []
            for s in range(NSTRIP):
                q_tile = resident.tile([128, 2, 2, STRIP], fp8, tag=f"q{s}", name=f"q{s}")
                k_tile = resident.tile([128, 2, STRIP], fp8, tag=f"k{s}", name=f"k{s}")
                v_tile = resident.tile([128, 4, HG, DK + 1], bf16, tag=f"v{s}", name=f"v{s}")
                nc.vector.memset(v_tile[:, :, :, DK : DK + 1], 1.0)
                o_tile = resident.tile([128, 2, STRIP], bf16, tag=f"o{s}", name=f"o{s}")
                q_s.append(q_tile); k_s.append(k_tile); v_s.append(v_tile); o_s.append(o_tile)

            def dr_products(ps, w_hi, w_lohi, x_t, msl, n_free):
                """3-product fp8 DoubleRow accumulation into psum ps.

                hi*hi over c-pairs, then (x_hi*w_lo + x_lo*w_hi) per c."""
                first = True
                for j in range(4):
                    nc.tensor.matmul(ps, lhsT=w_hi[:, 2 * j : 2 * j + 2, msl],
                                     rhs=x_t[:, 2 * j : 2 * j + 2, 0, :],
                                     start=first, stop=False, perf_mode=DR)
                    first = False
                for c in range(8):
                    nc.tensor.matmul(ps, lhsT=w_lohi[:, c, :, msl],
                                     rhs=x_t[:, c, :, :],
                                     start=False, stop=(c == 7), perf_mode=DR)

            def proj_qk(nm, t):
                """Project q or k substrip t into fp8 (q: hi/lo, k: single)."""
                ctx_ = nc.named_scope(f"proj{nm}{t}"); ctx_.__enter__()
                x_t = x_first.pop((nm, t), None)
                if x_t is None:
                    x_t = load_x(nm, t)
                biasP = bqP if nm == "q" else bkP
                wk_ = "w" + nm
                for m in range(2):
                    ps = bank.tile([128, SUB], f32, tag="bank")
                    msl = slice(m * 128, (m + 1) * 128)
                    dr_products(ps, w_h[wk_], w_x[wk_], x_t, msl, SUB)
                    if nm == "k":
                        dst = k_s[t][:, m, :]
                        if qkv_bias:
                            nc.vector.tensor_scalar(out=dst, in0=ps,
                                                    scalar1=1.0 / WS,
                                                    scalar2=biasP[:, m : m + 1],
                                                    op0=MULT, op1=ADD)
                        else:
                            nc.vector.tensor_scalar_mul(dst, ps, 1.0 / WS)
                    else:
                        hi = q_s[t][:, m, 0, :]
                        lo = q_s[t][:, m, 1, :]
                        if qkv_bias:
                            qtf = qtmp.tile([128, SUB], f32, tag="qtmp")
                            nc.vector.tensor_scalar(out=qtf, in0=ps,
                                                    scalar1=1.0 / WS,
                                                    scalar2=biasP[:, m : m + 1],
                                                    op0=MULT, op1=ADD)
                            nc.vector.tensor_copy(hi, qtf)
                            nc.vector.tensor_sub(lo, qtf, hi)
                        else:
                            nc.vector.tensor_scalar_mul(hi, ps, 1.0 / WS)
                            nc.vector.scalar_tensor_tensor(
                                out=lo, in0=ps, scalar=1.0 / WS, in1=hi,
                                op0=MULT, op1=mybir.AluOpType.subtract)
                ctx_.__exit__(None, None, None)

            def proj_q(t):
                proj_qk("q", t)

            def proj_k(t):
                proj_qk("k", t)

            def proj_v(t):
                ctx_ = nc.named_scope(f"projv{t}"); ctx_.__enter__()
                x_t = x_first.pop(("v", t), None)
                if x_t is None:
                    x_t = load_x("v", t)
                for j in range(4):
                    ps = bank.tile([128, DH], f32, tag="bank")
                    psl = slice(j * 128, (j + 1) * 128)
                    if qkv_bias:
                        nc.tensor.matmul(ps, lhsT=ones_b, rhs=bv_t,
                                         start=True, stop=False)
                    first = not qkv_bias
                    for jj in range(4):
                        nc.tensor.matmul(ps, lhsT=x_t[:, 2 * jj : 2 * jj + 2, 0, psl],
                                         rhs=w_h["wv"][:, 2 * jj : 2 * jj + 2, :],
                                         start=first, stop=False, perf_mode=DR)
                        first = False
                    for c in range(8):
                        # cross: lhsT slots (x_hi, x_lo), rhs slots (w_lo, w_hi)
                        nc.tensor.matmul(ps, lhsT=x_t[:, c, :, psl],
                                         rhs=w_x["wv"][:, c, :, :],
                                         start=False, stop=(c == 7), perf_mode=DR)
                    nc.vector.tensor_scalar_mul(
                        v_s[t][:, j, :, 0:DK],
                        ps.rearrange("p (h d) -> p h d", h=HG), 1.0 / WS)
                ctx_.__exit__(None, None, None)

            def proj_kv(t):
                proj_k(t)
                proj_v(t)

            def attention_hp(s, hp):
                ctx_ = nc.named_scope(f"attn{s}h{hp}"); ctx_.__enter__()
                q0 = s * STRIP
                a_max = 4 * s + 3 if causal else NKT - 1
                if True:
                    o_ps0 = bank.tile([65, STRIP], f32, tag="bank")
                    o_ps1 = bank.tile([65, STRIP], f32, tag="bank")
                    o_ps = [o_ps0, o_ps1]
                    for a in range(a_max + 1):
                        lo = max((a - 4 * s) * KT, 0) if causal else 0
                        sp = sps.tile([128, 2, STRIP], f32, tag="sps")
                        for i in range(2):
                            pr = slice(i * 64, (i + 1) * 64)
                            kt_sl = slice((a % 4) * KT, (a % 4 + 1) * KT)
                            nc.tensor.matmul(
                                sp[:, i, lo:STRIP],
                                lhsT=k_s[a // 4][pr, hp, kt_sl]
                                    .unsqueeze(1).broadcast_to([64, 2, KT]),
                                rhs=q_s[s][pr, hp, :, lo:STRIP],
                                start=True, stop=True, perf_mode=DR,
                            )
                        pt = ptile.tile([128, 2, STRIP], bf16, tag="pt")
                        nc.scalar.activation(out=pt[:, :, lo:STRIP],
                                             in_=sp[:, :, lo:STRIP],
                                             func=EXP, scale=0.125)
                        if causal and 0 <= a - 4 * s <= 3:
                            d0 = (a - 4 * s) * KT
                            for i in range(2):
                                nc.vector.tensor_mul(pt[:, i, d0:d0 + KT],
                                                     pt[:, i, d0:d0 + KT], tri_t)
                        if not causal:
                            mt = mtile.tile([128, STRIP], bf16, tag="mt")
                            nc.sync.dma_start(
                                out=mt, in_=maskT[a * KT:(a + 1) * KT, q0:q0 + STRIP])
                            for i in range(2):
                                nc.vector.tensor_mul(pt[:, i, :], pt[:, i, :], mt)
                        for i in range(2):
                            nc.tensor.matmul(o_ps[i][:, lo:STRIP],
                                             lhsT=v_s[a // 4][:, a % 4, 2 * hp + i, :],
                                             rhs=pt[:, i, lo:STRIP],
                                             start=(a == 0), stop=(a == a_max))
                    # normalize: recip of denom row, broadcast over 64
                    # partitions.  Last strip: per-quarter so wo/st/out
                    # drain incrementally behind the final PV.
                    quarters = ((slice(0, STRIP),) if s < NSTRIP - 1 else
                                tuple(slice(t4 * KT, (t4 + 1) * KT) for t4 in range(4)))
                    for i in range(2):
                        for qsl in quarters:
                            qn = qsl.stop - qsl.start
                            r_t = small.tile([1, STRIP], f32r, tag="recip")
                            with nc.allow_low_precision(reason="float32r is fp32 bits"):
                                if causal:
                                    nc.vector.reciprocal(r_t[:, 0:qn], o_ps[i][64:65, qsl])
                                else:
                                    dn = small.tile([1, STRIP], f32r, tag="denom")
                                    nc.vector.tensor_scalar_max(dn[:, 0:qn], o_ps[i][64:65, qsl], 1e-30)
                                    nc.vector.reciprocal(r_t[:, 0:qn], dn[:, 0:qn])
                            bc_ps = bank.tile([64, STRIP], f32, tag="bank")
                            nc.tensor.matmul(bc_ps[:, 0:qn], lhsT=ones_r[0:1, 0:64],
                                             rhs=r_t[:, 0:qn], start=True, stop=True)
                            bc_t = small.tile([64, STRIP], f32, tag="bc")
                            nc.vector.tensor_copy(bc_t[:, 0:qn], bc_ps[:, 0:qn])
                            nc.vector.tensor_mul(
                                o_s[s][i * 64:(i + 1) * 64, hp, qsl],
                                o_ps[i][0:64, qsl], bc_t[:, 0:qn])

                ctx_.__exit__(None, None, None)

            def wo_strip(s):
                ctx_ = nc.named_scope(f"wo{s}"); ctx_.__enter__()
                st = stage.tile([128, 4, D], f16, tag="st")
                for t4 in range(4):
                    csl = slice(t4 * 128, (t4 + 1) * 128)
                    for n in range(2):
                        wps = bank.tile([128, 512], f32, tag="bank")
                        nsl = slice(n * 512, (n + 1) * 512)
                        for c in range(2):
                            nc.tensor.matmul(wps, lhsT=o_s[s][:, c, csl],
                                             rhs=wo_t[:, c, nsl],
                                             start=(c == 0), stop=(c == 1))
                        if s == NSTRIP - 1 and (t4 + n) % 2 == 0:
                            nc.scalar.copy(out=st[:, t4, nsl], in_=wps)
                        else:
                            nc.vector.tensor_copy(st[:, t4, nsl], wps)
                for t4 in range(4):
                    r0 = s * STRIP + t4 * 128
                    nc.sync.dma_start(
                        out=out[r0:r0 + 128, :],
                        in_=st[:, t4, :],
                    )
                ctx_.__exit__(None, None, None)

            if causal:
                proj_q(0)
                proj_kv(0)
                proj_q(1)
                proj_kv(1)
                with tc.high_priority():
                    attention_hp(0, 0)
                proj_q(2)
                with tc.high_priority():
                    attention_hp(0, 1)
                proj_k(2)
                with tc.high_priority():
                    attention_hp(1, 0)
                wo_strip(0)
                proj_v(2)
                proj_q(3)
                with tc.high_priority():
                    attention_hp(1, 1)
                proj_k(3)
                with tc.high_priority():
                    attention_hp(2, 0)
                proj_v(3)
                with tc.high_priority():
                    attention_hp(2, 1)
                wo_strip(1)
                with tc.high_priority():
                    attention_hp(3, 0)
                wo_strip(2)
                with tc.high_priority():
                    attention_hp(3, 1)
                wo_strip(3)
            else:
                for t in range(NSUB):
                    proj_q(t)
                    proj_kv(t)
                for s in range(NSTRIP):
                    attention_hp(s, 0)
                    attention_hp(s, 1)
                    wo_strip(s)

    nc.compile()
    return nc


def _get_kernel(causal: bool, qkv_bias: bool):
    key = ("attn", causal, qkv_bias)
    if key not in _CACHE:
        _CACHE[key] = _build(causal, qkv_bias)
    return _CACHE[key]


def _split8(x):
    import ml_dtypes

    e4 = ml_dtypes.float8_e4m3
    hi = np.ascontiguousarray(x).astype(e4)
    lo = np.ascontiguousarray(x - hi.astype(np.float32)).astype(e4)
    return hi, lo


def kernel(query, key, value, mask, wq, bq, wk, bk, wv, bv, wo, bo):
    import ml_dtypes
    from concourse import bass_utils

    f32 = np.float32
    bf16 = ml_dtypes.bfloat16

    mask_b = np.asarray(mask, dtype=bool)
    causal = bool(
        (mask_b[:, 0] == np.tril(np.ones((L, L), dtype=bool))[None]).all()
    )
    qkv_bias = bool(np.any(np.asarray(bq)) or np.any(np.asarray(bk))
                    or np.any(np.asarray(bv)))
    nc = _get_kernel(causal, qkv_bias)

    tri_np = np.triu(np.ones((KT, KT), dtype=f32)).astype(bf16)
    xsplit = {}
    for nm, tns in (("q", query), ("k", key), ("v", value)):
        for b in range(B):
            xT = np.ascontiguousarray(np.asarray(tns[b], f32).T)
            xsplit[(nm, b)] = _split8(xT)
    if not causal:
        maskT = [
            np.ascontiguousarray(mask_b[b, 0].T).astype(bf16) for b in range(B)
        ]

    wq = np.asarray(wq, f32)
    wk = np.asarray(wk, f32)
    wv = np.asarray(wv, f32)
    wo = np.asarray(wo, f32)
    bq = np.asarray(bq, f32)
    bk = np.asarray(bk, f32)
    bv = np.asarray(bv, f32)

    in_maps = []
    for c in range(NCORES):
        b, g = c // TP, c % TP
        gs = slice(g * DH, (g + 1) * DH)
        m = {"tri": tri_np}
        for nm in ("q", "k", "v"):
            m[nm + "Th"], m[nm + "Tl"] = xsplit[(nm, b)]
        for nm, w in (("wq", wq), ("wk", wk), ("wv", wv)):
            wh, wl = _split8(np.ascontiguousarray(w[:, gs]) * WS)
            m[nm + "h"] = np.ascontiguousarray(
                wh.reshape(8, 128, DH).transpose(1, 0, 2).reshape(128, 8 * DH))
            # interleaved (lo, hi) pairs flattened to [D, 2*DH]
            m[nm + "x"] = np.ascontiguousarray(
                np.stack([wl, wh], axis=1).reshape(D, 2 * DH))
        m["wo"] = np.ascontiguousarray(wo[gs, :]).astype(bf16)
        m["bq"] = np.ascontiguousarray(bq[gs])
        m["bk"] = np.ascontiguousarray(bk[gs])
        m["bv"] = (np.ascontiguousarray(bv[gs]) * WS).astype(bf16)
        if not causal:
            m["maskT"] = maskT[b]
        in_maps.append(m)

    res = bass_utils.run_bass_kernel_spmd(nc, in_maps, core_ids=list(range(NCORES)))

    out = np.zeros((B, L, D), f32)
    for c in range(NCORES):
        out[c // TP] += res.results[c]["out"].astype(f32)
    out += np.asarray(bo, f32)[None, None, :]
    return out


# revision 25
# speedup vs baseline: 1.0913x; 1.0913x over previous
"""Multi-head causal attention (B=2, L=2048, D=1024, H=16) on 8 trn2 cores.

Sharding: data-parallel over batch (2) x tensor-parallel over heads (4 groups
of 4 heads).  Core c handles batch c//4, heads 4*(c%4) .. 4*(c%4)+3.
Wq/Wk/Wv are column-sharded, Wo row-sharded; the TP all-reduce after Wo is
done host-side (sum of the 4 partial outputs per batch), as is the bo add.

Per-core kernel:
  - QKV projections run in fp8e4 DoubleRow perf mode: the host supplies
    X^T split into e4m3 hi/lo residual streams (same total bytes as bf16)
    and W scaled by 256 split hi/lo; the three products hi*hi + hi*lo +
    lo*hi are computed with DoubleRow matmuls (two 128-contraction chunks
    per instruction at 0.5 cycles/row), 25% cheaper than bf16 and at
    bf16-level accuracy.  The 1/256 unscale + bias folds into the
    PSUM->SBUF copy.
  - Q^T/K^T are quantized to e4m3 and shuffled (SBUF->SBUF DMA) into a
    [32, 2(dk-half), head, n] layout so S^T = K^T tile ^T Q^T runs as a
    single fp8 DoubleRow matmul per (key-tile, head) - half the bf16 cost.
  - softmax without max-subtraction (scores bounded); exp on ACT emits
    P^T bf16 directly in the layout PV needs; a ones-row appended to V
    makes the PV matmul also emit softmax denominators
  - normalization: reciprocal of the denom row, partition-broadcast via a
    K=1 matmul (ones x recip), multiplied into O^T before the Wo matmul
  - causal mask: S^T/exp/PV restricted to q >= k-tile start; the single
    diagonal 128x128 block is masked by multiplying P^T with an upper-
    triangular 0/1 tile (supplied as input)
  - two heads are processed per S^T psum tile so each exp instruction
    covers [128, 2, 512-lo] (ACT per-instruction overhead is ~370ns)
"""

import numpy as np

B, L, D, H = 2, 2048, 1024, 16
DK = D // H          # 64
NCORES = 8
TP = 4               # head-group shards per batch
HG = H // TP         # 4 heads per core
DH = HG * DK         # 256 per-core head dims
STRIP = 512          # attention q-strip width
NSTRIP = L // STRIP  # 4
SUB = 512            # projection substrip width
NSUB = L // SUB      # 4
KT = 128             # key tile
NKT = L // KT        # 16
WS = 256.0           # fp8 weight pre-scale (undone in PSUM->SBUF copies)

_CACHE = {}


def _build(causal: bool, qkv_bias: bool = True):
    import concourse.mybir as mybir
    import concourse.tile as tile
    from concourse import bacc

    f32 = mybir.dt.float32
    f32r = mybir.dt.float32r
    bf16 = mybir.dt.bfloat16
    fp8 = mybir.dt.float8e4
    f16 = mybir.dt.float16
    EXP = mybir.ActivationFunctionType.Exp
    DR = mybir.MatmulPerfMode.DoubleRow
    MULT = mybir.AluOpType.mult
    ADD = mybir.AluOpType.add

    nc = bacc.Bacc("TRN2", target_bir_lowering=False)

    # hi/lo fp8 input streams (X^T, [D, L])
    xs = {}
    for nm in ("q", "k", "v"):
        xs[nm + "h"] = nc.dram_tensor(nm + "Th", [D, L], fp8, kind="ExternalInput")
        xs[nm + "l"] = nc.dram_tensor(nm + "Tl", [D, L], fp8, kind="ExternalInput")
    # weights: wh = e4m3(W*WS) [D, DH]; wx = (lo, hi) interleaved [D, 2*DH]
    ws = {}
    for nm in ("wq", "wk", "wv"):
        ws[nm + "h"] = nc.dram_tensor(nm + "h", [128, 8 * DH], fp8, kind="ExternalInput")
        ws[nm + "x"] = nc.dram_tensor(nm + "x", [D, 2 * DH], fp8, kind="ExternalInput")
    wo = nc.dram_tensor("wo", [DH, D], bf16, kind="ExternalInput")
    bq = nc.dram_tensor("bq", [DH], f32, kind="ExternalInput")
    bk = nc.dram_tensor("bk", [DH], f32, kind="ExternalInput")
    bv = nc.dram_tensor("bv", [DH], bf16, kind="ExternalInput")  # pre-scaled by WS
    tri = nc.dram_tensor("tri", [KT, KT], bf16, kind="ExternalInput")
    maskT = None
    if not causal:
        maskT = nc.dram_tensor("maskT", [L, L], bf16, kind="ExternalInput")
    out = nc.dram_tensor("out", [L, D], f16, kind="ExternalOutput")

    with tile.TileContext(nc) as tc:
        with (
            tc.tile_pool(name="consts", bufs=1) as consts,
            tc.tile_pool(name="resident", bufs=1) as resident,
            tc.tile_pool(name="xin", bufs=2) as xin,
            tc.tile_pool(name="qtmp", bufs=2) as qtmp,
            tc.tile_pool(name="ptile", bufs=18) as ptile,
            tc.tile_pool(name="small", bufs=2) as small,
            tc.tile_pool(name="stage", bufs=2) as stage,
            tc.tile_pool(name="mtile", bufs=4) as mtile,
            tc.tile_pool(name="bank", bufs=4, space="PSUM") as bank,
            tc.tile_pool(name="sps", bufs=2, space="PSUM") as sps,
        ):
            # ---- weights / constants (q,k streams first, v later) ----
            w_h, w_x = {}, {}
            for nm in ("wq", "wk", "wv"):
                w_h[nm] = consts.tile([128, 8, DH], fp8, tag=nm + "h", name=nm + "h")
                w_x[nm] = consts.tile([128, 8, 2, DH], fp8, tag=nm + "x", name=nm + "x")
            bqP = consts.tile([128, 2], f32, tag="bqP")
            bkP = consts.tile([128, 2], f32, tag="bkP")
            nc.sync.dma_start(out=bqP, in_=bq.rearrange("(m p) -> p m", p=128))
            nc.sync.dma_start(out=bkP, in_=bk.rearrange("(m p) -> p m", p=128))
            bv_t = consts.tile([1, DH], bf16, tag="bv")
            nc.sync.dma_start(out=bv_t, in_=bv[:].unsqueeze(0))
            tri_t = consts.tile([KT, KT], bf16, tag="tri")
            nc.sync.dma_start(out=tri_t, in_=tri[:])
            ones_f = consts.tile([1, 128], f32, tag="ones")
            nc.vector.memset(ones_f, 1.0)
            ones_r = ones_f[:, :].bitcast(f32r)
            ones_b = consts.tile([1, 128], bf16, tag="onesb")
            nc.vector.memset(ones_b, 1.0)
            wo_t = consts.tile([128, 2, D], bf16, tag="wo")
            nc.sync.dma_start(out=wo_t, in_=wo.rearrange("(c p) n -> p c n", p=128))

            # ---- input substrip tiles: [128, 8(c), 2(hi/lo), SUB] fp8 ----
            def load_x(nm, t, first=None):
                if first is not None:
                    x_t = first
                else:
                    x_t = xin.tile([128, 8, 2, SUB], fp8, tag="x" + nm)
                sl = slice(t * SUB, (t + 1) * SUB)
                nc.sync.dma_start(out=x_t[:, :, 0, :], in_=xs[nm + "h"][:, sl].rearrange("(c p) n -> p c n", p=128))
                nc.sync.dma_start(out=x_t[:, :, 1, :], in_=xs[nm + "l"][:, sl].rearrange("(c p) n -> p c n", p=128))
                return x_t

            xa = xin.tile([128, 8, 2, SUB], fp8, tag="xq", name="xq_pre0")
            xb = xin.tile([128, 8, 2, SUB], fp8, tag="xk", name="xk_pre0")
            xc0 = xin.tile([128, 8, 2, SUB], fp8, tag="xv", name="xv_pre0")
            x_first = {("q", 0): xa, ("k", 0): xb, ("v", 0): xc0}

            def _ldw(nm, xnm, part):
                xt = x_first[(xnm, 0)]
                if part == 0:
                    nc.sync.dma_start(out=w_h[nm], in_=ws[nm + "h"].rearrange("p (c m) -> p c m", c=8))
                    nc.sync.dma_start(out=xt[:, :, 0, :], in_=xs[xnm + "h"][:, 0:SUB].rearrange("(c p) n -> p c n", p=128))
                else:
                    nc.sync.dma_start(out=w_x[nm], in_=ws[nm + "x"].rearrange("(c p) (u m) -> p c u m", p=128, u=2))
                    nc.sync.dma_start(out=xt[:, :, 1, :], in_=xs[xnm + "l"][:, 0:SUB].rearrange("(c p) n -> p c n", p=128))

            _ldw("wq", "q", 0); _ldw("wk", "k", 0)
            _ldw("wq", "q", 1); _ldw("wk", "k", 1)
            _ldw("wv", "v", 0); _ldw("wv", "v", 1)

            # ---- resident activations ----
            # q8: [128(=head-half*64+dk), 2(m-chunk), 2(hi/lo), n] fp8
            # k8: [128, 2(m-chunk), n] fp8 (single)
            q_s, k_s, v_s, o_s = [], [], [], []
            for s in range(NSTRIP):
                q_tile = resident.tile([128, 2, 2, STRIP], fp8, tag=f"q{s}", name=f"q{s}")
                k_tile = resident.tile([128, 2, STRIP], fp8, tag=f"k{s}", name=f"k{s}")
                v_tile = resident.tile([128, 4, HG, DK + 1], bf16, tag=f"v{s}", name=f"v{s}")
                nc.vector.memset(v_tile[:, :, :, DK : DK + 1], 1.0)
                o_tile = resident.tile([128, 2, STRIP], bf16, tag=f"o{s}", name=f"o{s}")
                q_s.append(q_tile); k_s.append(k_tile); v_s.append(v_tile); o_s.append(o_tile)

            def dr_products(ps, w_hi, w_lohi, x_t, msl, n_free):
                """3-product fp8 DoubleRow accumulation into psum ps.

                hi*hi over c-pairs, then (x_hi*w_lo + x_lo*w_hi) per c."""
                first = True
                for j in range(4):
                    nc.tensor.matmul(ps, lhsT=w_hi[:, 2 * j : 2 * j + 2, msl],
                                     rhs=x_t[:, 2 * j : 2 * j + 2, 0, :],
                                     start=first, stop=False, perf_mode=DR)
                    first = False
                for c in range(8):
                    nc.tensor.matmul(ps, lhsT=w_lohi[:, c, :, msl],
                                     rhs=x_t[:, c, :, :],
                                     start=False, stop=(c == 7), perf_mode=DR)

            def proj_qk(nm, t):
                """Project q or k substrip t into fp8 (q: hi/lo, k: single)."""
                ctx_ = nc.named_scope(f"proj{nm}{t}"); ctx_.__enter__()
                x_t = x_first.pop((nm, t), None)
                if x_t is None:
                    x_t = load_x(nm, t)
                biasP = bqP if nm == "q" else bkP
                wk_ = "w" + nm
                for m in range(2):
                    ps = bank.tile([128, SUB], f32, tag="bank")
                    msl = slice(m * 128, (m + 1) * 128)
                    dr_products(ps, w_h[wk_], w_x[wk_], x_t, msl, SUB)
                    if nm == "k":
                        dst = k_s[t][:, m, :]
                        if qkv_bias:
                            nc.vector.tensor_scalar(out=dst, in0=ps,
                                                    scalar1=1.0 / WS,
                                                    scalar2=biasP[:, m : m + 1],
                                                    op0=MULT, op1=ADD)
                        else:
                            nc.vector.tensor_scalar_mul(dst, ps, 1.0 / WS)
                    else:
                        hi = q_s[t][:, m, 0, :]
                        lo = q_s[t][:, m, 1, :]
                        if qkv_bias:
                            qtf = qtmp.tile([128, SUB], f32, tag="qtmp")
                            nc.vector.tensor_scalar(out=qtf, in0=ps,
                                                    scalar1=1.0 / WS,
                                                    scalar2=biasP[:, m : m + 1],
                                                    op0=MULT, op1=ADD)
                            nc.vector.tensor_copy(hi, qtf)
                            nc.vector.tensor_sub(lo, qtf, hi)
                        else:
                            nc.vector.tensor_scalar_mul(hi, ps, 1.0 / WS)
                            nc.vector.scalar_tensor_tensor(
                                out=lo, in0=ps, scalar=1.0 / WS, in1=hi,
                                op0=MULT, op1=mybir.AluOpType.subtract)
                ctx_.__exit__(None, None, None)

            def proj_q(t):
                proj_qk("q", t)

            def proj_k(t):
                proj_qk("k", t)

            def proj_v(t):
                ctx_ = nc.named_scope(f"projv{t}"); ctx_.__enter__()
                x_t = x_first.pop(("v", t), None)
                if x_t is None:
                    x_t = load_x("v", t)
                for j in range(4):
                    ps = bank.tile([128, DH], f32, tag="bank")
                    psl = slice(j * 128, (j + 1) * 128)
                    if qkv_bias:
                        nc.tensor.matmul(ps, lhsT=ones_b, rhs=bv_t,
                                         start=True, stop=False)
                    first = not qkv_bias
                    for jj in range(4):
                        nc.tensor.matmul(ps, lhsT=x_t[:, 2 * jj : 2 * jj + 2, 0, psl],
                                         rhs=w_h["wv"][:, 2 * jj : 2 * jj + 2, :],
                                         start=first, stop=False, perf_mode=DR)
                        first = False
                    for c in range(8):
                        # cross: lhsT slots (x_hi, x_lo), rhs slots (w_lo, w_hi)
                        nc.tensor.matmul(ps, lhsT=x_t[:, c, :, psl],
                                         rhs=w_x["wv"][:, c, :, :],
                                         start=False, stop=(c == 7), perf_mode=DR)
                    nc.vector.tensor_scalar_mul(
                        v_s[t][:, j, :, 0:DK],
                        ps.rearrange("p (h d) -> p h d", h=HG), 1.0 / WS)
                ctx_.__exit__(None, None, None)

            def proj_kv(t):
                proj_k(t)
                proj_v(t)

            def attention_hp(s, hp):
                ctx_ = nc.named_scope(f"attn{s}h{hp}"); ctx_.__enter__()
                q0 = s * STRIP
                a_max = 4 * s + 3 if causal else NKT - 1
                if True:
                    o_ps0 = bank.tile([65, STRIP], f32, tag="bank")
                    o_ps1 = bank.tile([65, STRIP], f32, tag="bank")
                    o_ps = [o_ps0, o_ps1]
                    for a in range(a_max + 1):
                        lo = max((a - 4 * s) * KT, 0) if causal else 0
                        sp = sps.tile([128, 2, STRIP], f32, tag="sps")
                        for i in range(2):
                            pr = slice(i * 64, (i + 1) * 64)
                            kt_sl = slice((a % 4) * KT, (a % 4 + 1) * KT)
                            nc.tensor.matmul(
                                sp[:, i, lo:STRIP],
                                lhsT=k_s[a // 4][pr, hp, kt_sl]
                                    .unsqueeze(1).broadcast_to([64, 2, KT]),
                                rhs=q_s[s][pr, hp, :, lo:STRIP],
                                start=True, stop=True, perf_mode=DR,
                            )
                        pt = ptile.tile([128, 2, STRIP], bf16, tag="pt")
                        nc.scalar.activation(out=pt[:, :, lo:STRIP],
                                             in_=sp[:, :, lo:STRIP],
                                             func=EXP, scale=0.125)
                        if causal and 0 <= a - 4 * s <= 3:
                            d0 = (a - 4 * s) * KT
                            for i in range(2):
                                nc.vector.tensor_mul(pt[:, i, d0:d0 + KT],
                                                     pt[:, i, d0:d0 + KT], tri_t)
                        if not causal:
                            mt = mtile.tile([128, STRIP], bf16, tag="mt")
                            nc.sync.dma_start(
                                out=mt, in_=maskT[a * KT:(a + 1) * KT, q0:q0 + STRIP])
                            for i in range(2):
                                nc.vector.tensor_mul(pt[:, i, :], pt[:, i, :], mt)
                        for i in range(2):
                            nc.tensor.matmul(o_ps[i][:, lo:STRIP],
                                             lhsT=v_s[a // 4][:, a % 4, 2 * hp + i, :],
                                             rhs=pt[:, i, lo:STRIP],
                                             start=(a == 0), stop=(a == a_max))
                    # normalize: recip of denom row, broadcast over 64 partitions
                    for i in range(2):
                        r_t = small.tile([1, STRIP], f32r, tag="recip")
                        with nc.allow_low_precision(reason="float32r is fp32 bits"):
                            if causal:
                                nc.vector.reciprocal(r_t, o_ps[i][64:65, :])
                            else:
                                dn = small.tile([1, STRIP], f32r, tag="denom")
                                nc.vector.tensor_scalar_max(dn, o_ps[i][64:65, :], 1e-30)
                                nc.vector.reciprocal(r_t, dn)
                        bc_ps = bank.tile([64, STRIP], f32, tag="bank")
                        nc.tensor.matmul(bc_ps, lhsT=ones_r[0:1, 0:64], rhs=r_t,
                                         start=True, stop=True)
                        bc_t = small.tile([64, STRIP], f32, tag="bc")
                        nc.vector.tensor_copy(bc_t, bc_ps)
                        nc.vector.tensor_mul(
                            o_s[s][i * 64:(i + 1) * 64, hp, :],
                            o_ps[i][0:64, :], bc_t)

                ctx_.__exit__(None, None, None)

            def wo_strip(s):
                ctx_ = nc.named_scope(f"wo{s}"); ctx_.__enter__()
                st = stage.tile([128, 4, D], f16, tag="st")
                for t4 in range(4):
                    csl = slice(t4 * 128, (t4 + 1) * 128)
                    for n in range(2):
                        wps = bank.tile([128, 512], f32, tag="bank")
                        nsl = slice(n * 512, (n + 1) * 512)
                        for c in range(2):
                            nc.tensor.matmul(wps, lhsT=o_s[s][:, c, csl],
                                             rhs=wo_t[:, c, nsl],
                                             start=(c == 0), stop=(c == 1))
                        if s == NSTRIP - 1 and (t4 + n) % 2 == 0:
                            nc.scalar.copy(out=st[:, t4, nsl], in_=wps)
                        else:
                            nc.vector.tensor_copy(st[:, t4, nsl], wps)
                for t4 in range(4):
                    r0 = s * STRIP + t4 * 128
                    nc.sync.dma_start(
                        out=out[r0:r0 + 128, :],
                        in_=st[:, t4, :],
                    )
                ctx_.__exit__(None, None, None)

            if causal:
                proj_q(0)
                proj_kv(0)
                proj_q(1)
                proj_kv(1)
                with tc.high_priority():
                    attention_hp(0, 0)
                proj_q(2)
                with tc.high_priority():
                    attention_hp(0, 1)
                proj_k(2)
                with tc.high_priority():
                    attention_hp(1, 0)
                wo_strip(0)
                proj_v(2)
                proj_q(3)
                with tc.high_priority():
                    attention_hp(1, 1)
                proj_k(3)
                with tc.high_priority():
                    attention_hp(2, 0)
                proj_v(3)
                with tc.high_priority():
                    attention_hp(2, 1)
                wo_strip(1)
                with tc.high_priority():
                    attention_hp(3, 0)
                wo_strip(2)
                with tc.high_priority():
                    attention_hp(3, 1)
                wo_strip(3)
            else:
                for t in range(NSUB):
                    proj_q(t)
                    proj_kv(t)
                for s in range(NSTRIP):
                    attention_hp(s, 0)
                    attention_hp(s, 1)
                    wo_strip(s)

    nc.compile()
    return nc


def _get_kernel(causal: bool, qkv_bias: bool):
    key = ("attn", causal, qkv_bias)
    if key not in _CACHE:
        _CACHE[key] = _build(causal, qkv_bias)
    return _CACHE[key]


def _split8(x):
    import ml_dtypes

    e4 = ml_dtypes.float8_e4m3
    hi = np.ascontiguousarray(x).astype(e4)
    lo = np.ascontiguousarray(x - hi.astype(np.float32)).astype(e4)
    return hi, lo


def kernel(query, key, value, mask, wq, bq, wk, bk, wv, bv, wo, bo):
    import ml_dtypes
    from concourse import bass_utils

    f32 = np.float32
    bf16 = ml_dtypes.bfloat16

    mask_b = np.asarray(mask, dtype=bool)
    causal = bool(
        (mask_b[:, 0] == np.tril(np.ones((L, L), dtype=bool))[None]).all()
    )
    qkv_bias = bool(np.any(np.asarray(bq)) or np.any(np.asarray(bk))
                    or np.any(np.asarray(bv)))
    nc = _get_kernel(causal, qkv_bias)

    tri_np = np.triu(np.ones((KT, KT), dtype=f32)).astype(bf16)
    xsplit = {}
    for nm, tns in (("q", query), ("k", key), ("v", value)):
        for b in range(B):
            xT = np.ascontiguousarray(np.asarray(tns[b], f32).T)
            xsplit[(nm, b)] = _split8(xT)
    if not causal:
        maskT = [
            np.ascontiguousarray(mask_b[b, 0].T).astype(bf16) for b in range(B)
        ]

    wq = np.asarray(wq, f32)
    wk = np.asarray(wk, f32)
    wv = np.asarray(wv, f32)
    wo = np.asarray(wo, f32)
    bq = np.asarray(bq, f32)
    bk = np.asarray(bk, f32)
    bv = np.asarray(bv, f32)

    in_maps = []
    for c in range(NCORES):
        b, g = c // TP, c % TP
        gs = slice(g * DH, (g + 1) * DH)
        m = {"tri": tri_np}
        for nm in ("q", "k", "v"):
            m[nm + "Th"], m[nm + "Tl"] = xsplit[(nm, b)]
        for nm, w in (("wq", wq), ("wk", wk), ("wv", wv)):
            wh, wl = _split8(np.ascontiguousarray(w[:, gs]) * WS)
            m[nm + "h"] = np.ascontiguousarray(
                wh.reshape(8, 128, DH).transpose(1, 0, 2).reshape(128, 8 * DH))
            # interleaved (lo, hi) pairs flattened to [D, 2*DH]
            m[nm + "x"] = np.ascontiguousarray(
                np.stack([wl, wh], axis=1).reshape(D, 2 * DH))
        m["wo"] = np.ascontiguousarray(wo[gs, :]).astype(bf16)
        m["bq"] = np.ascontiguousarray(bq[gs])
        m["bk"] = np.ascontiguousarray(bk[gs])
        m["bv"] = (np.ascontiguousarray(bv[gs]) * WS).astype(bf16)
        if not causal:
            m["maskT"] = maskT[b]
        in_maps.append(m)

    res = bass_utils.run_bass_kernel_spmd(nc, in_maps, core_ids=list(range(NCORES)))

    out = np.zeros((B, L, D), f32)
    for c in range(NCORES):
        out[c // TP] += res.results[c]["out"].astype(f32)
    out += np.asarray(bo, f32)[None, None, :]
    return out


# revision 26
# speedup vs baseline: 1.0947x; 1.0031x over previous
"""Multi-head causal attention (B=2, L=2048, D=1024, H=16) on 8 trn2 cores.

Sharding: data-parallel over batch (2) x tensor-parallel over heads (4 groups
of 4 heads).  Core c handles batch c//4, heads 4*(c%4) .. 4*(c%4)+3.
Wq/Wk/Wv are column-sharded, Wo row-sharded; the TP all-reduce after Wo is
done host-side (sum of the 4 partial outputs per batch), as is the bo add.

Per-core kernel:
  - QKV projections run in fp8e4 DoubleRow perf mode: the host supplies
    X^T split into e4m3 hi/lo residual streams (same total bytes as bf16)
    and W scaled by 256 split hi/lo; the three products hi*hi + hi*lo +
    lo*hi are computed with DoubleRow matmuls (two 128-contraction chunks
    per instruction at 0.5 cycles/row), 25% cheaper than bf16 and at
    bf16-level accuracy.  The 1/256 unscale + bias folds into the
    PSUM->SBUF copy.
  - Q^T/K^T are quantized to e4m3 and shuffled (SBUF->SBUF DMA) into a
    [32, 2(dk-half), head, n] layout so S^T = K^T tile ^T Q^T runs as a
    single fp8 DoubleRow matmul per (key-tile, head) - half the bf16 cost.
  - softmax without max-subtraction (scores bounded); exp on ACT emits
    P^T bf16 directly in the layout PV needs; a ones-row appended to V
    makes the PV matmul also emit softmax denominators
  - normalization: reciprocal of the denom row, partition-broadcast via a
    K=1 matmul (ones x recip), multiplied into O^T before the Wo matmul
  - causal mask: S^T/exp/PV restricted to q >= k-tile start; the single
    diagonal 128x128 block is masked by multiplying P^T with an upper-
    triangular 0/1 tile (supplied as input)
  - two heads are processed per S^T psum tile so each exp instruction
    covers [128, 2, 512-lo] (ACT per-instruction overhead is ~370ns)
"""

import numpy as np

B, L, D, H = 2, 2048, 1024, 16
DK = D // H          # 64
NCORES = 8
TP = 4               # head-group shards per batch
HG = H // TP         # 4 heads per core
DH = HG * DK         # 256 per-core head dims
STRIP = 512          # attention q-strip width
NSTRIP = L // STRIP  # 4
SUB = 512            # projection substrip width
NSUB = L // SUB      # 4
KT = 128             # key tile
NKT = L // KT        # 16
WS = 256.0           # fp8 weight pre-scale (undone in PSUM->SBUF copies)

_CACHE = {}


def _build(causal: bool, qkv_bias: bool = True):
    import concourse.mybir as mybir
    import concourse.tile as tile
    from concourse import bacc

    f32 = mybir.dt.float32
    f32r = mybir.dt.float32r
    bf16 = mybir.dt.bfloat16
    fp8 = mybir.dt.float8e4
    f16 = mybir.dt.float16
    EXP = mybir.ActivationFunctionType.Exp
    DR = mybir.MatmulPerfMode.DoubleRow
    MULT = mybir.AluOpType.mult
    ADD = mybir.AluOpType.add

    nc = bacc.Bacc("TRN2", target_bir_lowering=False)

    # hi/lo fp8 input streams (X^T, [D, L])
    xs = {}
    for nm in ("q", "k", "v"):
        xs[nm + "h"] = nc.dram_tensor(nm + "Th", [D, L], fp8, kind="ExternalInput")
        xs[nm + "l"] = nc.dram_tensor(nm + "Tl", [D, L], fp8, kind="ExternalInput")
    # weights: wh = e4m3(W*WS) [D, DH]; wx = (lo, hi) interleaved [D, 2*DH]
    ws = {}
    for nm in ("wq", "wk", "wv"):
        ws[nm + "h"] = nc.dram_tensor(nm + "h", [128, 8 * DH], fp8, kind="ExternalInput")
        ws[nm + "x"] = nc.dram_tensor(nm + "x", [D, 2 * DH], fp8, kind="ExternalInput")
    wo = nc.dram_tensor("wo", [DH, D], bf16, kind="ExternalInput")
    bq = nc.dram_tensor("bq", [DH], f32, kind="ExternalInput")
    bk = nc.dram_tensor("bk", [DH], f32, kind="ExternalInput")
    bv = nc.dram_tensor("bv", [DH], bf16, kind="ExternalInput")  # pre-scaled by WS
    tri = nc.dram_tensor("tri", [KT, KT], bf16, kind="ExternalInput")
    maskT = None
    if not causal:
        maskT = nc.dram_tensor("maskT", [L, L], bf16, kind="ExternalInput")
    out = nc.dram_tensor("out", [L, D], f16, kind="ExternalOutput")

    with tile.TileContext(nc) as tc:
        with (
            tc.tile_pool(name="consts", bufs=1) as consts,
            tc.tile_pool(name="resident", bufs=1) as resident,
            tc.tile_pool(name="xin", bufs=2) as xin,
            tc.tile_pool(name="qtmp", bufs=2) as qtmp,
            tc.tile_pool(name="ptile", bufs=18) as ptile,
            tc.tile_pool(name="small", bufs=2) as small,
            tc.tile_pool(name="stage", bufs=2) as stage,
            tc.tile_pool(name="mtile", bufs=4) as mtile,
            tc.tile_pool(name="bank", bufs=4, space="PSUM") as bank,
            tc.tile_pool(name="sps", bufs=2, space="PSUM") as sps,
        ):
            # ---- weights / constants (q,k streams first, v later) ----
            w_h, w_x = {}, {}
            for nm in ("wq", "wk", "wv"):
                w_h[nm] = consts.tile([128, 8, DH], fp8, tag=nm + "h", name=nm + "h")
                w_x[nm] = consts.tile([128, 8, 2, DH], fp8, tag=nm + "x", name=nm + "x")
            bqP = consts.tile([128, 2], f32, tag="bqP")
            bkP = consts.tile([128, 2], f32, tag="bkP")
            nc.sync.dma_start(out=bqP, in_=bq.rearrange("(m p) -> p m", p=128))
            nc.sync.dma_start(out=bkP, in_=bk.rearrange("(m p) -> p m", p=128))
            bv_t = consts.tile([1, DH], bf16, tag="bv")
            nc.sync.dma_start(out=bv_t, in_=bv[:].unsqueeze(0))
            tri_t = consts.tile([KT, KT], bf16, tag="tri")
            nc.sync.dma_start(out=tri_t, in_=tri[:])
            ones_f = consts.tile([1, 128], f32, tag="ones")
            nc.vector.memset(ones_f, 1.0)
            ones_r = ones_f[:, :].bitcast(f32r)
            ones_b = consts.tile([1, 128], bf16, tag="onesb")
            nc.vector.memset(ones_b, 1.0)
            wo_t = consts.tile([128, 2, D], bf16, tag="wo")
            nc.sync.dma_start(out=wo_t, in_=wo.rearrange("(c p) n -> p c n", p=128))

            # ---- input substrip tiles: [128, 8(c), 2(hi/lo), SUB] fp8 ----
            def load_x(nm, t, first=None):
                if first is not None:
                    x_t = first
                else:
                    x_t = xin.tile([128, 8, 2, SUB], fp8, tag="x" + nm)
                sl = slice(t * SUB, (t + 1) * SUB)
                nc.sync.dma_start(out=x_t[:, :, 0, :], in_=xs[nm + "h"][:, sl].rearrange("(c p) n -> p c n", p=128))
                nc.sync.dma_start(out=x_t[:, :, 1, :], in_=xs[nm + "l"][:, sl].rearrange("(c p) n -> p c n", p=128))
                return x_t

            xa = xin.tile([128, 8, 2, SUB], fp8, tag="xq", name="xq_pre0")
            xb = xin.tile([128, 8, 2, SUB], fp8, tag="xk", name="xk_pre0")
            xc0 = xin.tile([128, 8, 2, SUB], fp8, tag="xv", name="xv_pre0")
            x_first = {("q", 0): xa, ("k", 0): xb, ("v", 0): xc0}

            def _ldw(nm, xnm, part):
                xt = x_first[(xnm, 0)]
                if part == 0:
                    nc.sync.dma_start(out=w_h[nm], in_=ws[nm + "h"].rearrange("p (c m) -> p c m", c=8))
                    nc.sync.dma_start(out=xt[:, :, 0, :], in_=xs[xnm + "h"][:, 0:SUB].rearrange("(c p) n -> p c n", p=128))
                else:
                    nc.sync.dma_start(out=w_x[nm], in_=ws[nm + "x"].rearrange("(c p) (u m) -> p c u m", p=128, u=2))
                    nc.sync.dma_start(out=xt[:, :, 1, :], in_=xs[xnm + "l"][:, 0:SUB].rearrange("(c p) n -> p c n", p=128))

            _ldw("wq", "q", 0); _ldw("wk", "k", 0)
            _ldw("wq", "q", 1); _ldw("wk", "k", 1)
            _ldw("wv", "v", 0); _ldw("wv", "v", 1)

            # ---- resident activations ----
            # q8: [128(=head-half*64+dk), 2(m-chunk), 2(hi/lo), n] fp8
            # k8: [128, 2(m-chunk), n] fp8 (single)
            q_s, k_s, v_s, o_s = [], [], [], []
            for s in range(NSTRIP):
                q_tile = resident.tile([128, 2, 2, STRIP], fp8, tag=f"q{s}", name=f"q{s}")
                k_tile = resident.tile([128, 2, STRIP], fp8, tag=f"k{s}", name=f"k{s}")
                v_tile = resident.tile([128, 4, HG, DK + 1], bf16, tag=f"v{s}", name=f"v{s}")
                nc.vector.memset(v_tile[:, :, :, DK : DK + 1], 1.0)
                o_tile = resident.tile([128, 2, STRIP], bf16, tag=f"o{s}", name=f"o{s}")
                q_s.append(q_tile); k_s.append(k_tile); v_s.append(v_tile); o_s.append(o_tile)

            def dr_products(ps, w_hi, w_lohi, x_t, msl, n_free):
                """3-product fp8 DoubleRow accumulation into psum ps.

                hi*hi over c-pairs, then (x_hi*w_lo + x_lo*w_hi) per c."""
                first = True
                for j in range(4):
                    nc.tensor.matmul(ps, lhsT=w_hi[:, 2 * j : 2 * j + 2, msl],
                                     rhs=x_t[:, 2 * j : 2 * j + 2, 0, :],
                                     start=first, stop=False, perf_mode=DR)
                    first = False
                for c in range(8):
                    nc.tensor.matmul(ps, lhsT=w_lohi[:, c, :, msl],
                                     rhs=x_t[:, c, :, :],
                                     start=False, stop=(c == 7), perf_mode=DR)

            def proj_qk(nm, t):
                """Project q or k substrip t into fp8 (q: hi/lo, k: single)."""
                ctx_ = nc.named_scope(f"proj{nm}{t}"); ctx_.__enter__()
                x_t = x_first.pop((nm, t), None)
                if x_t is None:
                    x_t = load_x(nm, t)
                biasP = bqP if nm == "q" else bkP
                wk_ = "w" + nm
                for m in range(2):
                    ps = bank.tile([128, SUB], f32, tag="bank")
                    msl = slice(m * 128, (m + 1) * 128)
                    dr_products(ps, w_h[wk_], w_x[wk_], x_t, msl, SUB)
                    if nm == "k":
                        dst = k_s[t][:, m, :]
                        if qkv_bias:
                            nc.vector.tensor_scalar(out=dst, in0=ps,
                                                    scalar1=1.0 / WS,
                                                    scalar2=biasP[:, m : m + 1],
                                                    op0=MULT, op1=ADD)
                        else:
                            nc.vector.tensor_scalar_mul(dst, ps, 1.0 / WS)
                    else:
                        hi = q_s[t][:, m, 0, :]
                        lo = q_s[t][:, m, 1, :]
                        if qkv_bias:
                            qtf = qtmp.tile([128, SUB], f32, tag="qtmp")
                            nc.vector.tensor_scalar(out=qtf, in0=ps,
                                                    scalar1=1.0 / WS,
                                                    scalar2=biasP[:, m : m + 1],
                                                    op0=MULT, op1=ADD)
                            nc.vector.tensor_copy(hi, qtf)
                            nc.vector.tensor_sub(lo, qtf, hi)
                        else:
                            nc.vector.tensor_scalar_mul(hi, ps, 1.0 / WS)
                            nc.vector.scalar_tensor_tensor(
                                out=lo, in0=ps, scalar=1.0 / WS, in1=hi,
                                op0=MULT, op1=mybir.AluOpType.subtract)
                ctx_.__exit__(None, None, None)

            def proj_q(t):
                proj_qk("q", t)

            def proj_k(t):
                proj_qk("k", t)

            def proj_v(t):
                ctx_ = nc.named_scope(f"projv{t}"); ctx_.__enter__()
                x_t = x_first.pop(("v", t), None)
                if x_t is None:
                    x_t = load_x("v", t)
                for j in range(4):
                    ps = bank.tile([128, DH], f32, tag="bank")
                    psl = slice(j * 128, (j + 1) * 128)
                    if qkv_bias:
                        nc.tensor.matmul(ps, lhsT=ones_b, rhs=bv_t,
                                         start=True, stop=False)
                    first = not qkv_bias
                    for jj in range(4):
                        nc.tensor.matmul(ps, lhsT=x_t[:, 2 * jj : 2 * jj + 2, 0, psl],
                                         rhs=w_h["wv"][:, 2 * jj : 2 * jj + 2, :],
                                         start=first, stop=False, perf_mode=DR)
                        first = False
                    for c in range(8):
                        # cross: lhsT slots (x_hi, x_lo), rhs slots (w_lo, w_hi)
                        nc.tensor.matmul(ps, lhsT=x_t[:, c, :, psl],
                                         rhs=w_x["wv"][:, c, :, :],
                                         start=False, stop=(c == 7), perf_mode=DR)
                    nc.vector.tensor_scalar_mul(
                        v_s[t][:, j, :, 0:DK],
                        ps.rearrange("p (h d) -> p h d", h=HG), 1.0 / WS)
                ctx_.__exit__(None, None, None)

            def proj_kv(t):
                proj_k(t)
                proj_v(t)

            def attention_hp(s, hp):
                ctx_ = nc.named_scope(f"attn{s}h{hp}"); ctx_.__enter__()
                q0 = s * STRIP
                a_max = 4 * s + 3 if causal else NKT - 1
                if True:
                    o_ps0 = bank.tile([65, STRIP], f32, tag="bank")
                    o_ps1 = bank.tile([65, STRIP], f32, tag="bank")
                    o_ps = [o_ps0, o_ps1]
                    for a in range(a_max + 1):
                        lo = max((a - 4 * s) * KT, 0) if causal else 0
                        sp = sps.tile([128, 2, STRIP], f32, tag="sps")
                        for i in range(2):
                            pr = slice(i * 64, (i + 1) * 64)
                            kt_sl = slice((a % 4) * KT, (a % 4 + 1) * KT)
                            nc.tensor.matmul(
                                sp[:, i, lo:STRIP],
                                lhsT=k_s[a // 4][pr, hp, kt_sl]
                                    .unsqueeze(1).broadcast_to([64, 2, KT]),
                                rhs=q_s[s][pr, hp, :, lo:STRIP],
                                start=True, stop=True, perf_mode=DR,
                            )
                        pt = ptile.tile([128, 2, STRIP], bf16, tag="pt")
                        nc.scalar.activation(out=pt[:, :, lo:STRIP],
                                             in_=sp[:, :, lo:STRIP],
                                             func=EXP, scale=0.125)
                        if causal and 0 <= a - 4 * s <= 3:
                            d0 = (a - 4 * s) * KT
                            for i in range(2):
                                nc.vector.tensor_mul(pt[:, i, d0:d0 + KT],
                                                     pt[:, i, d0:d0 + KT], tri_t)
                        if not causal:
                            mt = mtile.tile([128, STRIP], bf16, tag="mt")
                            nc.sync.dma_start(
                                out=mt, in_=maskT[a * KT:(a + 1) * KT, q0:q0 + STRIP])
                            for i in range(2):
                                nc.vector.tensor_mul(pt[:, i, :], pt[:, i, :], mt)
                        for i in range(2):
                            nc.tensor.matmul(o_ps[i][:, lo:STRIP],
                                             lhsT=v_s[a // 4][:, a % 4, 2 * hp + i, :],
                                             rhs=pt[:, i, lo:STRIP],
                                             start=(a == 0), stop=(a == a_max))
                    # normalize: recip of denom row, broadcast over 64 partitions
                    for i in range(2):
                        r_t = small.tile([1, STRIP], f32r, tag="recip")
                        with nc.allow_low_precision(reason="float32r is fp32 bits"):
                            if causal:
                                nc.vector.reciprocal(r_t, o_ps[i][64:65, :])
                            else:
                                dn = small.tile([1, STRIP], f32r, tag="denom")
                                nc.vector.tensor_scalar_max(dn, o_ps[i][64:65, :], 1e-30)
                                nc.vector.reciprocal(r_t, dn)
                        bc_ps = bank.tile([64, STRIP], f32, tag="bank")
                        nc.tensor.matmul(bc_ps, lhsT=ones_r[0:1, 0:64], rhs=r_t,
                                         start=True, stop=True)
                        bc_t = small.tile([64, STRIP], f32, tag="bc")
                        nc.vector.tensor_copy(bc_t, bc_ps)
                        nc.vector.tensor_mul(
                            o_s[s][i * 64:(i + 1) * 64, hp, :],
                            o_ps[i][0:64, :], bc_t)

                ctx_.__exit__(None, None, None)

            def wo_strip(s):
                ctx_ = nc.named_scope(f"wo{s}"); ctx_.__enter__()
                st = stage.tile([128, 4, D], f16, tag="st")
                for t4 in range(4):
                    csl = slice(t4 * 128, (t4 + 1) * 128)
                    for n in range(2):
                        wps = bank.tile([128, 512], f32, tag="bank")
                        nsl = slice(n * 512, (n + 1) * 512)
                        for c in range(2):
                            nc.tensor.matmul(wps, lhsT=o_s[s][:, c, csl],
                                             rhs=wo_t[:, c, nsl],
                                             start=(c == 0), stop=(c == 1))
                        if s == NSTRIP - 1 and (t4 + n) % 2 == 0:
                            nc.scalar.copy(out=st[:, t4, nsl], in_=wps)
                        else:
                            nc.vector.tensor_copy(st[:, t4, nsl], wps)
                for t4 in range(4):
                    r0 = s * STRIP + t4 * 128
                    nc.sync.dma_start(
                        out=out[r0:r0 + 128, :],
                        in_=st[:, t4, :],
                    )
                ctx_.__exit__(None, None, None)

            if causal:
                proj_q(0)
                proj_kv(0)
                proj_q(1)
                proj_kv(1)
                with tc.high_priority():
                    attention_hp(0, 0)
                proj_q(2)
                with tc.high_priority():
                    attention_hp(0, 1)
                proj_k(2)
                proj_v(2)
                with tc.high_priority():
                    attention_hp(1, 0)
                wo_strip(0)
                proj_q(3)
                with tc.high_priority():
                    attention_hp(1, 1)
                proj_k(3)
                proj_v(3)
                with tc.high_priority():
                    attention_hp(2, 0)
                with tc.high_priority():
                    attention_hp(2, 1)
                wo_strip(1)
                with tc.high_priority():
                    attention_hp(3, 0)
                wo_strip(2)
                with tc.high_priority():
                    attention_hp(3, 1)
                wo_strip(3)
            else:
                for t in range(NSUB):
                    proj_q(t)
                    proj_kv(t)
                for s in range(NSTRIP):
                    attention_hp(s, 0)
                    attention_hp(s, 1)
                    wo_strip(s)

    nc.compile()
    return nc


def _get_kernel(causal: bool, qkv_bias: bool):
    key = ("attn", causal, qkv_bias)
    if key not in _CACHE:
        _CACHE[key] = _build(causal, qkv_bias)
    return _CACHE[key]


def _split8(x):
    import ml_dtypes

    e4 = ml_dtypes.float8_e4m3
    hi = np.ascontiguousarray(x).astype(e4)
    lo = np.ascontiguousarray(x - hi.astype(np.float32)).astype(e4)
    return hi, lo


def kernel(query, key, value, mask, wq, bq, wk, bk, wv, bv, wo, bo):
    import ml_dtypes
    from concourse import bass_utils

    f32 = np.float32
    bf16 = ml_dtypes.bfloat16

    mask_b = np.asarray(mask, dtype=bool)
    causal = bool(
        (mask_b[:, 0] == np.tril(np.ones((L, L), dtype=bool))[None]).all()
    )
    qkv_bias = bool(np.any(np.asarray(bq)) or np.any(np.asarray(bk))
                    or np.any(np.asarray(bv)))
    nc = _get_kernel(causal, qkv_bias)

    tri_np = np.triu(np.ones((KT, KT), dtype=f32)).astype(bf16)
    xsplit = {}
    for nm, tns in (("q", query), ("k", key), ("v", value)):
        for b in range(B):
            xT = np.ascontiguousarray(np.asarray(tns[b], f32).T)
            xsplit[(nm, b)] = _split8(xT)
    if not causal:
        maskT = [
            np.ascontiguousarray(mask_b[b, 0].T).astype(bf16) for b in range(B)
        ]

    wq = np.asarray(wq, f32)
    wk = np.asarray(wk, f32)
    wv = np.asarray(wv, f32)
    wo = np.asarray(wo, f32)
    bq = np.asarray(bq, f32)
    bk = np.asarray(bk, f32)
    bv = np.asarray(bv, f32)

    in_maps = []
    for c in range(NCORES):
        b, g = c // TP, c % TP
        gs = slice(g * DH, (g + 1) * DH)
        m = {"tri": tri_np}
        for nm in ("q", "k", "v"):
            m[nm + "Th"], m[nm + "Tl"] = xsplit[(nm, b)]
        for nm, w in (("wq", wq), ("wk", wk), ("wv", wv)):
            wh, wl = _split8(np.ascontiguousarray(w[:, gs]) * WS)
            m[nm + "h"] = np.ascontiguousarray(
                wh.reshape(8, 128, DH).transpose(1, 0, 2).reshape(128, 8 * DH))
            # interleaved (lo, hi) pairs flattened to [D, 2*DH]
            m[nm + "x"] = np.ascontiguousarray(
                np.stack([wl, wh], axis=1).reshape(D, 2 * DH))
        m["wo"] = np.ascontiguousarray(wo[gs, :]).astype(bf16)
        m["bq"] = np.ascontiguousarray(bq[gs])
        m["bk"] = np.ascontiguousarray(bk[gs])
        m["bv"] = (np.ascontiguousarray(bv[gs]) * WS).astype(bf16)
        if not causal:
            m["maskT"] = maskT[b]
        in_maps.append(m)

    res = bass_utils.run_bass_kernel_spmd(nc, in_maps, core_ids=list(range(NCORES)))

    out = np.zeros((B, L, D), f32)
    for c in range(NCORES):
        out[c // TP] += res.results[c]["out"].astype(f32)
    out += np.asarray(bo, f32)[None, None, :]
    return out
